# revision 1
# baseline (speedup 1.0000x reference)
"""MeshRefineNet GNN (4 GraphConv layers) on 8 TRN2 NeuronCores via Bass/Tile.

Strategy (graph-parallel, SPMD):
  * Vertices (320000 real + 512 dummies) are packed into 8 cores x 313
    groups x 128 slots with a balanced binning so that every
    (core, group, 32-slot subgroup) bin receives at most 256 incident
    half-edges.  All activations live in this permuted layout; the host
    un-permutes at the end.
  * Aggregation per group: gather the half-edge source rows (8 tiles of
    128 rows) from a core-local bf16 replica of the activations with
    indirect DMA, build 0/1 selection matrices S from precomputed
    segment ranks with one vector is_equal op, and accumulate
    X_t^T @ S_t into PSUM (feature-major) on the TensorEngine.
  * Transform per group: psum2 = XA^T @ W1 + x^T @ W0 (+ x0^T @ I for
    the residual at layer 2), ReLU on the ScalarEngine, bf16 store.
  * Between layers, an 8-core AllGather rebuilds the full replica from
    the per-core shards.  Layer 3 produces the [*, 3] output in f32.
Biases are all zero in this problem's setup; if nonzero biases are ever
passed, a numpy fallback computes the exact reference instead.
"""
import sys
import numpy as np

if "/opt/trn_rl_repo" not in sys.path:
    sys.path.insert(0, "/opt/trn_rl_repo")

P = 128
SUBW = 32
SUBS = 4
G_TILES = 8
CAP = 256
GB = 8


class Cfg:
    def __init__(self, N, E, NC=8, GROUPS=313):
        self.N, self.E, self.NC, self.GROUPS = N, E, NC, GROUPS
        self.SLOTS = GROUPS * 128
        self.TILES = GROUPS * G_TILES
        self.NBINS = NC * GROUPS * SUBS
        self.NREP = NC * self.SLOTS
        assert self.NREP >= N


CFG_FULL = Cfg(N=320000, E=960000, NC=8, GROUPS=313)


# ----------------------------------------------------------------- host prep
def build_tables(edges, cfg):
    N, NC, GROUPS = cfg.N, cfg.NC, cfg.GROUPS
    SLOTS, NBINS, TILES = cfg.SLOTS, cfg.NBINS, cfg.TILES

    src0 = edges[:, 0].astype(np.int64)
    dst0 = edges[:, 1].astype(np.int64)
    tgt = np.concatenate([src0, dst0])
    src = np.concatenate([dst0, src0])
    deg = np.bincount(tgt, minlength=N)

    n_items = NBINS * SUBW
    deg_ext = np.concatenate([deg, np.zeros(n_items - N, np.int64)])
    order = np.argsort(-deg_ext, kind="stable")
    grid = order.reshape(SUBW, NBINS).copy()
    for r in range(1, SUBW, 2):
        grid[r] = grid[r, ::-1]
    assert deg_ext[grid].sum(axis=0).max() <= CAP, "bin overflow"

    item_ids = grid.ravel()
    rr, bb = np.divmod(np.arange(SUBW * NBINS), NBINS)
    core_of = np.empty(n_items, np.int64)
    slot_of = np.empty(n_items, np.int64)
    core_of[item_ids] = bb // (GROUPS * SUBS)
    g_item = (bb % (GROUPS * SUBS)) // SUBS
    slot_of[item_ids] = g_item * 128 + (bb % SUBS) * SUBW + rr
    R = core_of * SLOTS + slot_of
    dummy_rep_row = int(R[N])

    vert_at = np.full((NC, SLOTS), -1, np.int64)
    vert_at[core_of[:N], slot_of[:N]] = np.arange(N)

    c_t, s_t = core_of[tgt], slot_of[tgt]
    g_t = s_t // 128
    rank = s_t % 128
    k_t = rank // SUBW
    binid = (c_t * GROUPS + g_t) * SUBS + k_t
    eorder = np.argsort(binid, kind="stable")
    sb = binid[eorder]
    pos = np.arange(sb.size) - np.searchsorted(sb, np.arange(NBINS))[sb]
    assert pos.max() < CAP
    he_core = c_t[eorder]
    he_tile = g_t[eorder] * G_TILES + k_t[eorder] * 2 + pos // 128
    he_row = pos % 128

    gidx = np.full((NC, 128, TILES), dummy_rep_row, np.int32)
    seg = np.zeros((NC, 128, TILES), np.float32)
    gidx[he_core, he_row, he_tile] = R[src[eorder]].astype(np.int32)
    seg[he_core, he_row, he_tile] = (rank % SUBW)[eorder].astype(np.float32)

    return dict(gidx=gidx, seg=seg, vert_at=vert_at)


def permute_rows(x, vert_at, cfg):
    out = np.zeros((cfg.NC, cfg.SLOTS, x.shape[1]), x.dtype)
    m = vert_at >= 0
    out[m] = x[vert_at[m]]
    return out


# ------------------------------------------------------------- device build
def _batches(groups):
    out, g0 = [], 0
    while g0 < groups:
        ng = min(GB, groups - g0)
        out.append((g0, ng))
        g0 += ng
    return out


def build_nc(cfg):
    import concourse.bacc as bacc
    import concourse.tile as tile
    import concourse.mybir as mybir
    from concourse.bass import IndirectOffsetOnAxis

    BF = mybir.dt.bfloat16
    F32 = mybir.dt.float32
    I32 = mybir.dt.int32
    RELU = mybir.ActivationFunctionType.Relu
    WCOLS = 6 * P + 6
    CCOLS = 256 + P

    NC_, GROUPS, SLOTS, TILES, NREP = (cfg.NC, cfg.GROUPS, cfg.SLOTS,
                                       cfg.TILES, cfg.NREP)

    nc = bacc.Bacc(None, target_bir_lowering=False, debug=False)
    xrep0 = nc.declare_dram_parameter("xrep0", [NREP, P], BF, isOutput=False)
    x0sh = nc.declare_dram_parameter("x0sh", [SLOTS, P], BF, isOutput=False)
    gidx = nc.declare_dram_parameter("gidx", [P, TILES], I32, isOutput=False)
    segr = nc.declare_dram_parameter("segr", [P, TILES], BF, isOutput=False)
    wts = nc.declare_dram_parameter("wts", [P, WCOLS], BF, isOutput=False)
    cst = nc.declare_dram_parameter("cst", [P, CCOLS], BF, isOutput=False)
    out = nc.declare_dram_parameter("out", [SLOTS, 3], F32, isOutput=True)

    xsh = {l: nc.dram_tensor(f"xsh{l}", [SLOTS, P], BF) for l in (1, 2, 3)}
    xrep = {l: nc.dram_tensor(f"xrep{l}", [NREP, P], BF, addr_space="Shared")
            for l in (1, 2, 3)}

    with tile.TileContext(nc) as tc:
        with (
            tc.tile_pool(name="res", bufs=1) as res,
            tc.tile_pool(name="gath", bufs=2) as gath_p,
            tc.tile_pool(name="xt", bufs=2) as xt_p,
            tc.tile_pool(name="x0t", bufs=2) as x0t_p,
            tc.tile_pool(name="s", bufs=4) as s_p,
            tc.tile_pool(name="xat", bufs=4) as xat_p,
            tc.tile_pool(name="ob", bufs=2) as ob_p,
            tc.tile_pool(name="psA", bufs=4, space="PSUM") as psA,
            tc.tile_pool(name="psB", bufs=4, space="PSUM") as psB,
        ):
            gidx_sb = res.tile([P, TILES], I32)
            nc.sync.dma_start(out=gidx_sb[:], in_=gidx.ap())
            segr_sb = res.tile([P, TILES], BF)
            nc.sync.dma_start(out=segr_sb[:], in_=segr.ap())
            wts_sb = res.tile([P, WCOLS], BF)
            nc.sync.dma_start(out=wts_sb[:], in_=wts.ap())
            cst_sb = res.tile([P, CCOLS], BF)
            nc.sync.dma_start(out=cst_sb[:], in_=cst.ap())

            iota_ap = cst_sb[:, 0:256].rearrange("p (a b) -> p a b", a=G_TILES)
            ident_ap = cst_sb[:, 256:256 + P]

            for layer in range(4):
                rep_ap = xrep0.ap() if layer == 0 else xrep[layer].ap()
                sh_ap = x0sh.ap() if layer == 0 else xsh[layer].ap()
                if layer < 3:
                    w1 = wts_sb[:, layer * 256: layer * 256 + P]
                    w0 = wts_sb[:, layer * 256 + P: layer * 256 + 2 * P]
                else:
                    w1 = wts_sb[:, 768:771]
                    w0 = wts_sb[:, 771:774]

                for (g0, ng) in _batches(GROUPS):
                    gbuf = gath_p.tile([P, GB * G_TILES, P], BF, tag="gbuf")
                    for tt in range(ng * G_TILES):
                        nc.gpsimd.indirect_dma_start(
                            out=gbuf[:, tt, :],
                            out_offset=None,
                            in_=rep_ap,
                            in_offset=IndirectOffsetOnAxis(
                                ap=gidx_sb[:, g0 * G_TILES + tt:
                                           g0 * G_TILES + tt + 1],
                                axis=0,
                            ),
                        )
                    xt = xt_p.tile([P, GB * P], BF, tag="xt")
                    nc.sync.dma_start(out=xt[:, 0:ng * P],
                                      in_=sh_ap[g0 * P:(g0 + ng) * P, :],
                                      transpose=True)
                    if layer == 2:
                        x0v = x0t_p.tile([P, GB, P], BF, tag="x0v")
                        nc.sync.dma_start(
                            out=x0v[:, 0:ng, :],
                            in_=x0sh.ap()[g0 * P:(g0 + ng) * P, :]
                                .rearrange("(g p) c -> p g c", p=P))
                    if layer < 3:
                        obuf = ob_p.tile([P, GB, P], BF, tag="ob")
                    else:
                        obuf = ob_p.tile([P, GB, 3], F32, tag="ob3")

                    for gl in range(ng):
                        g = g0 + gl
                        st = s_p.tile([P, G_TILES * SUBW], BF)
                        nc.vector.tensor_tensor(
                            out=st[:].rearrange("p (a b) -> p a b", a=G_TILES),
                            in0=segr_sb[:, g * G_TILES:(g + 1) * G_TILES]
                                .to_broadcast([P, G_TILES, SUBW]),
                            in1=iota_ap,
                            op=mybir.AluOpType.is_equal,
                        )
                        psumT = psA.tile([P, P], F32)
                        for t in range(G_TILES):
                            k = t // 2
                            nc.tensor.matmul(
                                psumT[:, k * SUBW:(k + 1) * SUBW],
                                lhsT=gbuf[:, gl * G_TILES + t, :],
                                rhs=st[:, t * SUBW:(t + 1) * SUBW],
                                start=(t % 2 == 0), stop=(t % 2 == 1),
                            )
                        xat = xat_p.tile([P, P], BF)
                        nc.vector.tensor_copy(out=xat[:], in_=psumT[:])

                        if layer < 3:
                            ps2 = psB.tile([P, P], F32, tag="ps2")
                            nc.tensor.matmul(ps2[:], lhsT=xat[:], rhs=w1,
                                             start=True, stop=False)
                            nc.tensor.matmul(ps2[:],
                                             lhsT=xt[:, gl * P:(gl + 1) * P],
                                             rhs=w0, start=False, stop=True)
                            if layer == 2:
                                rl = s_p.tile([P, P], BF, tag="rl")
                                nc.scalar.activation(out=rl[:], in_=ps2[:],
                                                     func=RELU)
                                nc.vector.tensor_add(out=obuf[:, gl, :],
                                                     in0=rl[:],
                                                     in1=x0v[:, gl, :])
                            else:
                                nc.scalar.activation(out=obuf[:, gl, :],
                                                     in_=ps2[:], func=RELU)
                        else:
                            ps2 = psB.tile([P, P], F32, tag="ps2")
                            nc.tensor.matmul(ps2[:, 0:3], lhsT=xat[:], rhs=w1,
                                             start=True, stop=False)
                            nc.tensor.matmul(ps2[:, 0:3],
                                             lhsT=xt[:, gl * P:(gl + 1) * P],
                                             rhs=w0, start=False, stop=True)
                            nc.vector.tensor_copy(out=obuf[:, gl, :],
                                                  in_=ps2[:, 0:3])

                    if layer < 3:
                        dst = xsh[layer + 1].ap()[g0 * P:(g0 + ng) * P, :] \
                            .rearrange("(g p) c -> p g c", p=P)
                    else:
                        dst = out.ap()[g0 * P:(g0 + ng) * P, :] \
                            .rearrange("(g p) c -> p g c", p=P)
                    nc.sync.dma_start(out=dst, in_=obuf[:, 0:ng, :])

                if layer < 3:
                    nc.gpsimd.collective_compute(
                        "AllGather", mybir.AluOpType.bypass,
                        replica_groups=[list(range(NC_))],
                        ins=[xsh[layer + 1].ap().opt()],
                        outs=[xrep[layer + 1].ap().opt()],
                    )
    nc.compile()
    return nc


# --------------------------------------------------------------- host driver
def _pack_inputs(inputs, tables, cfg):
    import ml_dtypes
    BF16 = ml_dtypes.bfloat16
    WCOLS = 6 * P + 6
    CCOLS = 256 + P

    feats = np.asarray(inputs["features"], np.float32)
    xsh0 = permute_rows(feats, tables["vert_at"], cfg).astype(BF16)
    xrep0 = np.ascontiguousarray(xsh0.reshape(cfg.NREP, P))

    wts = np.zeros((P, WCOLS), np.float32)
    for l in range(3):
        wts[:, l * 256:l * 256 + P] = np.asarray(inputs[f"W1_{l}"], np.float32)
        wts[:, l * 256 + P:l * 256 + 2 * P] = np.asarray(inputs[f"W0_{l}"],
                                                         np.float32)
    wts[:, 768:771] = np.asarray(inputs["W1_3"], np.float32)
    wts[:, 771:774] = np.asarray(inputs["W0_3"], np.float32)
    wts = wts.astype(BF16)

    cstn = np.zeros((P, CCOLS), np.float32)
    cstn[:, 0:256] = np.tile(np.arange(SUBW, dtype=np.float32), G_TILES)[None]
    cstn[:, 256:256 + P] = np.eye(P, dtype=np.float32)
    cstn = cstn.astype(BF16)

    in_maps = []
    for c in range(cfg.NC):
        in_maps.append({
            "xrep0": xrep0,
            "x0sh": np.ascontiguousarray(xsh0[c]),
            "gidx": np.ascontiguousarray(tables["gidx"][c]),
            "segr": np.ascontiguousarray(tables["seg"][c]).astype(BF16),
            "wts": wts,
            "cst": cstn,
        })
    return in_maps


def _np_fallback(inputs):
    x = np.asarray(inputs["features"], np.float32)
    e = np.asarray(inputs["edges"], np.int64)
    src, dst = e[:, 0], e[:, 1]

    def gc(x, i):
        h0 = x @ np.asarray(inputs[f"W0_{i}"], np.float32) + \
            np.asarray(inputs[f"b0_{i}"], np.float32)
        h1 = x @ np.asarray(inputs[f"W1_{i}"], np.float32) + \
            np.asarray(inputs[f"b1_{i}"], np.float32)
        agg = np.zeros_like(h0)
        np.add.at(agg, src, h1[dst])
        np.add.at(agg, dst, h1[src])
        return h0 + agg

    h = np.maximum(gc(x, 0), 0)
    h = np.maximum(gc(h, 1), 0)
    h = np.maximum(gc(h, 2), 0)
    h = h + x
    return gc(h, 3).reshape(2, 4, 40000, 3)


_NC_CACHE = {}
LAST_EXEC_TIME_NS = None


def kernel(**inputs):
    global LAST_EXEC_TIME_NS
    cfg = CFG_FULL
    feats = np.asarray(inputs["features"])
    assert feats.shape == (cfg.N, 128)

    for i in range(4):
        for b in ("b0", "b1"):
            v = inputs.get(f"{b}_{i}")
            if v is not None and np.any(np.asarray(v) != 0):
                return _np_fallback(inputs)

    from concourse.bass_utils import run_bass_kernel_spmd

    edges = np.asarray(inputs["edges"]).astype(np.int64)
    tables = build_tables(edges, cfg)
    if "nc" not in _NC_CACHE:
        _NC_CACHE["nc"] = build_nc(cfg)
    nc = _NC_CACHE["nc"]
    in_maps = _pack_inputs(inputs, tables, cfg)

    import os
    trace = bool(int(os.environ.get("GNN_TRACE", "0")))
    res = run_bass_kernel_spmd(nc, in_maps, core_ids=list(range(cfg.NC)),
                               trace=trace)
    LAST_EXEC_TIME_NS = res.exec_time_ns

    vert_at = tables["vert_at"]
    full = np.zeros((cfg.N, 3), np.float32)
    for c in range(cfg.NC):
        o = np.asarray(res.results[c]["out"], np.float32)
        m = vert_at[c] >= 0
        full[vert_at[c][m]] = o[m]
    return full.reshape(2, 4, 40000, 3)



# revision 5
# speedup vs baseline: 1.3911x; 1.3911x over previous
"""MeshRefineNet GNN on 8 TRN2 cores — dma_gather-based aggregation (v2).

v1 gathered neighbor rows with per-tile indirect DMAs (128 descriptors each,
~1us fixed SWDGE cost -> 10ms on GPSIMD).  v2 stages each batch's half-edge
rows with a few large dma_gather calls instead (no per-128-row fixed cost,
~8.5ns/descriptor Q7 generation), which
requires int16 indices: the replica is addressed through 10 windows of 32K
rows, and each batch's staged rows are sorted by (window, group, source row).
Tiles of 128 staged rows span at most 2 destination groups; a bf16 rank table
plus one is_equal builds the 0/1 scatter matrices S (width 256 = 2 group
blocks), and TensorE accumulates X_t^T @ S_t per group in PSUM.  The uniform
(max-over-cores) segment lengths keep the compiled program SPMD-identical;
per-core tables carry the indices/ranks.  Transforms, residual, and the
per-layer AllGather are unchanged from v1.
"""
import sys

if "/opt/trn_rl_repo" not in sys.path:
    sys.path.insert(0, "/opt/trn_rl_repo")

import numpy as np

P = 128
SUBW = 32
GB = 16            # groups per batch
WIN = 32768        # dma_gather int16 window (rows)
MAX_RUN = 8192     # split longer runs (descriptor-ring safety)


class Cfg:
    def __init__(self, N, E, NC=8, GROUPS=313):
        self.N, self.E, self.NC, self.GROUPS = N, E, NC, GROUPS
        self.SLOTS = GROUPS * 128
        self.NREP = NC * self.SLOTS
        self.NWIN = -(-self.NREP // WIN)
        self.NB = -(-GROUPS // GB)
        assert self.NREP >= N


CFG_FULL = Cfg(N=320000, E=960000, NC=8, GROUPS=313)


# ----------------------------------------------------------------- host prep
def build_tables(edges, cfg):
    N, NC, GROUPS = cfg.N, cfg.NC, cfg.GROUPS
    SLOTS, NB, NWIN = cfg.SLOTS, cfg.NB, cfg.NWIN

    src0 = edges[:, 0].astype(np.int64)
    dst0 = edges[:, 1].astype(np.int64)
    tgt = np.concatenate([src0, dst0])
    src = np.concatenate([dst0, src0])
    deg = np.bincount(tgt, minlength=N)

    # balanced vertex->slot assignment (degree snake over 32-row bins)
    NBINS = NC * GROUPS * 4
    n_items = NBINS * SUBW
    deg_ext = np.concatenate([deg, np.zeros(n_items - N, np.int64)])
    order = np.argsort(-deg_ext, kind="stable")
    grid = order.reshape(SUBW, NBINS).copy()
    for r in range(1, SUBW, 2):
        grid[r] = grid[r, ::-1]
    item_ids = grid.ravel()
    rr, bb = np.divmod(np.arange(SUBW * NBINS), NBINS)
    core_of = np.empty(n_items, np.int64)
    slot_of = np.empty(n_items, np.int64)
    core_of[item_ids] = bb // (GROUPS * 4)
    g_item = (bb % (GROUPS * 4)) // 4
    slot_of[item_ids] = g_item * 128 + (bb % 4) * SUBW + rr
    R = core_of * SLOTS + slot_of

    vert_at = np.full((NC, SLOTS), -1, np.int64)
    vert_at[core_of[:N], slot_of[:N]] = np.arange(N)

    c_t = core_of[tgt]
    s_t = slot_of[tgt]
    g_t = s_t // 128
    b_t = g_t // GB
    Rs = R[src]
    w_t = Rs // WIN

    # per (core, batch, window, group) counts -> uniform max lengths
    key = ((c_t * NB + b_t) * NWIN + w_t) * GROUPS + g_t
    cnt = np.bincount(key, minlength=NC * NB * NWIN * GROUPS) \
        .reshape(NC, NB, NWIN, GROUPS)
    Lmax = cnt.max(axis=0)          # [NB, NWIN, GROUPS]

    # ---- uniform walk: tiles / runs / mm schedule (core-independent) ----
    structure = []
    batchoff = []
    total = 0
    segstart = {}
    blockmap = {}                   # (tile_global, g) -> block index
    for b in range(NB):
        glo = b * GB
        ghi = min((b + 1) * GB, GROUPS)
        pos = 0
        runs = []
        tile_groups = []            # batch-local tile -> [g0, g1?]
        mms = []                    # (tile_local, block, g)
        for w in range(NWIN):
            run_start = pos
            for g in range(glo, ghi):
                L = int(Lmax[b, w, g])
                if L == 0:
                    continue
                t = pos // 128
                if pos % 128 != 0:
                    tg = tile_groups[t]
                    if g not in tg and len(tg) == 2:
                        pos = (t + 1) * 128
                segstart[(b, w, g)] = pos
                while L > 0:
                    t = pos // 128
                    while t >= len(tile_groups):
                        tile_groups.append([])
                    tg = tile_groups[t]
                    if g not in tg:
                        assert len(tg) < 2
                        tg.append(g)
                        mms.append((t, tg.index(g), g))
                        blockmap[(total // 128 + t, g)] = tg.index(g)
                    room = (t + 1) * 128 - pos
                    step = min(room, L)
                    pos += step
                    L -= step
            if pos % 128:
                pos = (pos // 128 + 1) * 128
            if pos > run_start:
                runs.append((w, run_start, pos - run_start))
        T_b = pos // 128
        # group-major MM lists (one PSUM bank session per group)
        gmms = {g: [] for g in range(glo, ghi)}
        for (t, blk, g) in mms:
            gmms[g].append((t, blk))
        for g in range(glo, ghi):
            assert gmms[g], f"group {g} empty in batch {b}"
        structure.append(dict(T=T_b, runs=runs, gmms=gmms,
                              glo=glo, ng=ghi - glo))
        batchoff.append(total)
        total += T_b * 128

    T_total = total // 128

    # ---- per-core tables: staged position of every half-edge ----
    # order within a segment: by (slot, source row)
    seg_of = ((c_t * NB + b_t) * NWIN + w_t) * GROUPS + g_t
    eorder = np.lexsort((Rs, s_t, seg_of))
    so = seg_of[eorder]
    first_in_seg = np.searchsorted(so, np.arange(NC * NB * NWIN * GROUPS))
    cum = np.arange(so.size) - first_in_seg[so]

    segstart_arr = np.zeros((NB, NWIN, GROUPS), np.int64)
    for (b, w, g), v in segstart.items():
        segstart_arr[b, w, g] = v
    boff_arr = np.asarray(batchoff, np.int64)

    et = eorder
    pos_local = segstart_arr[b_t[et], w_t[et], g_t[et]] + cum
    pos_global = boff_arr[b_t[et]] + pos_local
    tile_g = pos_global // 128
    part_g = pos_global % 128

    blk_of = np.zeros(et.size, np.int64)
    bm_keys = np.array([k[0] * GROUPS + k[1] for k in blockmap], np.int64)
    bm_vals = np.array(list(blockmap.values()), np.int64)
    bm = np.zeros(T_total * GROUPS, np.int64)
    bm[bm_keys] = bm_vals
    blk_of = bm[tile_g * GROUPS + g_t[et]]

    idx_lin = np.zeros((NC, total), np.int16)
    rank_lin = np.full((NC, total), 512.0, np.float32)
    cc = c_t[et]
    idx_lin[cc, pos_global] = (Rs[et] % WIN).astype(np.int16)
    rank_lin[cc, pos_global] = (s_t[et] % 128 + 128 * blk_of).astype(np.float32)

    # wrapped idx layout per run: local i -> (partition i%16 (+16k), col i//16)
    idx16 = np.zeros((NC, 128, total // 16), np.int16)
    for b in range(NB):
        for (w, start, L) in structure[b]["runs"]:
            s0 = batchoff[b] + start
            blkv = idx_lin[:, s0:s0 + L].reshape(NC, L // 16, 16) \
                .transpose(0, 2, 1)                       # [NC, 16, L/16]
            idx16[:, :, s0 // 16:(s0 + L) // 16] = np.tile(blkv, (1, 8, 1))

    rank_tbl = rank_lin.reshape(NC, T_total, 128).transpose(0, 2, 1).copy()

    return dict(idx16=idx16, rank=rank_tbl, vert_at=vert_at,
                structure=structure, batchoff=batchoff, T_total=T_total,
                T_max=max(s["T"] for s in structure))


def permute_rows(x, vert_at, cfg):
    out = np.zeros((cfg.NC, cfg.SLOTS, x.shape[1]), x.dtype)
    m = vert_at >= 0
    out[m] = x[vert_at[m]]
    return out


# ------------------------------------------------------------- device build
def build_nc(cfg, tables):
    import concourse.bacc as bacc
    import concourse.tile as tile
    import concourse.mybir as mybir
    from concourse import library_config

    BF = mybir.dt.bfloat16
    F32 = mybir.dt.float32
    I16 = mybir.dt.int16
    RELU = mybir.ActivationFunctionType.Relu
    WCOLS = 6 * P + 6

    NC_, GROUPS, SLOTS, NREP = cfg.NC, cfg.GROUPS, cfg.SLOTS, cfg.NREP
    structure = tables["structure"]
    batchoff = tables["batchoff"]
    T_total = tables["T_total"]
    T_max = tables["T_max"]
    HALF = -(-T_max // 2)

    nc = bacc.Bacc(None, target_bir_lowering=False, debug=False)
    xrep0 = nc.declare_dram_parameter("xrep0", [NREP, P], BF, isOutput=False)
    x0sh = nc.declare_dram_parameter("x0sh", [SLOTS, P], BF, isOutput=False)
    idx16 = nc.declare_dram_parameter("idx16", [P, T_total * 8], I16,
                                      isOutput=False)
    rankt = nc.declare_dram_parameter("rankt", [P, T_total], BF,
                                      isOutput=False)
    wts = nc.declare_dram_parameter("wts", [P, WCOLS], BF, isOutput=False)
    iot = nc.declare_dram_parameter("iot", [P, 256], BF, isOutput=False)
    out = nc.declare_dram_parameter("out", [SLOTS, 3], F32, isOutput=True)
    import os as _os
    DBG = bool(int(_os.environ.get("GNN_DBG", "0")))
    if DBG:
        T0 = tables["structure"][0]["T"]
        dbg_stg = nc.declare_dram_parameter("dbg_stg", [P, T0 * P], BF,
                                            isOutput=True)
        dbg_s = nc.declare_dram_parameter("dbg_s", [P, T0 * 256], BF,
                                          isOutput=True)


    xsh = {l: nc.dram_tensor(f"xsh{l}", [SLOTS, P], BF) for l in (1, 2, 3)}
    xrep = {l: nc.dram_tensor(f"xrep{l}", [NREP, P], BF, addr_space="Shared")
            for l in (1, 2, 3)}

    with tile.TileContext(nc) as tc:
        with (
            tc.tile_pool(name="res", bufs=1) as res,
            tc.tile_pool(name="idx", bufs=2) as idx_p,
            tc.tile_pool(name="stg", bufs=2) as stg_p,
            tc.tile_pool(name="s", bufs=2) as s_p,
            tc.tile_pool(name="xt", bufs=2) as xt_p,
            tc.tile_pool(name="x0t", bufs=2) as x0t_p,
            tc.tile_pool(name="xat", bufs=4) as xat_p,
            tc.tile_pool(name="ob", bufs=2) as ob_p,
            tc.tile_pool(name="psA", bufs=6, space="PSUM") as psA,
            tc.tile_pool(name="psB", bufs=2, space="PSUM") as psB,
        ):
            nc.gpsimd.load_library(library_config.attnmlp)
            rank_sb = res.tile([P, T_total], BF)
            nc.sync.dma_start(out=rank_sb[:], in_=rankt.ap())
            iota_sb = res.tile([P, HALF * 256], BF)
            nc.sync.dma_start(
                out=iota_sb[:].rearrange("p (t c) -> p t c", c=256),
                in_=iot.ap().rearrange("p (o c) -> p o c", o=1)
                    .to_broadcast([P, HALF, 256]))
            wts_sb = res.tile([P, WCOLS], BF)
            nc.sync.dma_start(out=wts_sb[:], in_=wts.ap())

            for layer in range(4):
                rep_ap = xrep0.ap() if layer == 0 else xrep[layer].ap()
                sh_ap = x0sh.ap() if layer == 0 else xsh[layer].ap()
                if layer < 3:
                    w1 = wts_sb[:, layer * 256: layer * 256 + P]
                    w0 = wts_sb[:, layer * 256 + P: layer * 256 + 2 * P]
                else:
                    w1 = wts_sb[:, 768:771]
                    w0 = wts_sb[:, 771:774]

                for b, st in enumerate(structure):
                    T_b, ng, glo = st["T"], st["ng"], st["glo"]
                    boff = batchoff[b]
                    bt0 = boff // 128

                    idxb = idx_p.tile([P, T_max * 8], I16, tag="idx")
                    nc.sync.dma_start(
                        out=idxb[:, 0:T_b * 8],
                        in_=idx16.ap()[:, boff // 16:(boff + T_b * 128) // 16])
                    stg = stg_p.tile([P, T_max, P], BF, tag="stg")
                    for (w, start, L) in st["runs"]:
                        w_hi = min((w + 1) * WIN, NREP)
                        o = 0
                        while o < L:
                            Lc = min(MAX_RUN, L - o)
                            nc.gpsimd.dma_gather(
                                out_ap=stg[:, (start + o) // 128:
                                           (start + o + Lc) // 128, :],
                                in_ap=rep_ap[w * WIN:w_hi, :],
                                idxs_ap=idxb[:, (start + o) // 16:
                                             (start + o + Lc) // 16],
                                num_idxs=Lc,
                                num_idxs_reg=Lc,
                                elem_size=P,
                                single_packet=False,
                            )
                            o += Lc

                    s_tiles = []
                    for hb in range(2):
                        t0 = hb * HALF
                        t1 = min(T_b, (hb + 1) * HALF)
                        if t0 >= t1:
                            s_tiles.append(None)
                            continue
                        S = s_p.tile([P, HALF * 256], BF, tag="s")
                        nc.vector.tensor_tensor(
                            out=S[:, 0:(t1 - t0) * 256]
                                .rearrange("p (t c) -> p t c", c=256),
                            in0=rank_sb[:, bt0 + t0:bt0 + t1]
                                .rearrange("p (t o) -> p t o", o=1)
                                .to_broadcast([P, t1 - t0, 256]),
                            in1=iota_sb[:, 0:(t1 - t0) * 256]
                                .rearrange("p (t c) -> p t c", c=256),
                            op=mybir.AluOpType.is_equal,
                        )
                        s_tiles.append((S, t0))

                    if DBG and layer == 0 and b == 0:
                        nc.sync.dma_start(
                            out=dbg_stg.ap(),
                            in_=stg[:, 0:T_b, :].rearrange("p a b -> p (a b)"))
                        for hb in range(2):
                            if s_tiles[hb] is None:
                                continue
                            S, t0 = s_tiles[hb]
                            t1 = min(T_b, t0 + HALF)
                            nc.sync.dma_start(
                                out=dbg_s.ap()[:, t0 * 256:t1 * 256],
                                in_=S[:, 0:(t1 - t0) * 256])

                    # ---- per-group aggregation + transforms ----
                    g0 = glo
                    xt = xt_p.tile([P, GB * P], BF, tag="xt")
                    nc.sync.dma_start(out=xt[:, 0:ng * P],
                                      in_=sh_ap[g0 * P:(g0 + ng) * P, :],
                                      transpose=True)
                    if layer == 2:
                        x0v = x0t_p.tile([P, GB, P], BF, tag="x0v")
                        nc.sync.dma_start(
                            out=x0v[:, 0:ng, :],
                            in_=x0sh.ap()[g0 * P:(g0 + ng) * P, :]
                                .rearrange("(g p) c -> p g c", p=P))
                    if layer < 3:
                        obuf = ob_p.tile([P, GB, P], BF, tag="ob")
                    else:
                        obuf = ob_p.tile([P, GB, 3], F32, tag="ob3")

                    for gl in range(ng):
                        glist = st["gmms"][glo + gl]
                        pg = psA.tile([P, P], F32, tag="pg")
                        for i, (t, blk) in enumerate(glist):
                            S, t0 = s_tiles[0] if t < HALF else s_tiles[1]
                            nc.tensor.matmul(
                                pg[:],
                                lhsT=stg[:, t, :],
                                rhs=S[:, (t - t0) * 256 + blk * 128:
                                      (t - t0) * 256 + blk * 128 + 128],
                                start=(i == 0), stop=(i == len(glist) - 1),
                            )
                        xat = xat_p.tile([P, P], BF)
                        nc.vector.tensor_copy(out=xat[:], in_=pg[:])
                        if layer < 3:
                            ps2 = psB.tile([P, P], F32, tag="ps2")
                            nc.tensor.matmul(ps2[:], lhsT=xat[:], rhs=w1,
                                             start=True, stop=False)
                            nc.tensor.matmul(ps2[:],
                                             lhsT=xt[:, gl * P:(gl + 1) * P],
                                             rhs=w0, start=False, stop=True)
                            if layer == 2:
                                rl = xat_p.tile([P, P], BF, tag="rl")
                                nc.scalar.activation(out=rl[:], in_=ps2[:],
                                                     func=RELU)
                                nc.vector.tensor_add(out=obuf[:, gl, :],
                                                     in0=rl[:],
                                                     in1=x0v[:, gl, :])
                            else:
                                nc.scalar.activation(out=obuf[:, gl, :],
                                                     in_=ps2[:], func=RELU)
                        else:
                            ps2 = psB.tile([P, P], F32, tag="ps2")
                            nc.tensor.matmul(ps2[:, 0:3], lhsT=xat[:], rhs=w1,
                                             start=True, stop=False)
                            nc.tensor.matmul(ps2[:, 0:3],
                                             lhsT=xt[:, gl * P:(gl + 1) * P],
                                             rhs=w0, start=False, stop=True)
                            nc.vector.tensor_copy(out=obuf[:, gl, :],
                                                  in_=ps2[:, 0:3])

                    if layer < 3:
                        dst = xsh[layer + 1].ap()[g0 * P:(g0 + ng) * P, :] \
                            .rearrange("(g p) c -> p g c", p=P)
                    else:
                        dst = out.ap()[g0 * P:(g0 + ng) * P, :] \
                            .rearrange("(g p) c -> p g c", p=P)
                    nc.sync.dma_start(out=dst, in_=obuf[:, 0:ng, :])

                if layer < 3:
                    nc.gpsimd.collective_compute(
                        "AllGather", mybir.AluOpType.bypass,
                        replica_groups=[list(range(NC_))],
                        ins=[xsh[layer + 1].ap().opt()],
                        outs=[xrep[layer + 1].ap().opt()],
                    )
    nc.compile()
    return nc


# --------------------------------------------------------------- host driver
def _pack_inputs(inputs, tables, cfg):
    import ml_dtypes
    BF16 = ml_dtypes.bfloat16
    WCOLS = 6 * P + 6

    feats = np.asarray(inputs["features"], np.float32)
    xsh0 = permute_rows(feats, tables["vert_at"], cfg).astype(BF16)
    xrep0 = np.ascontiguousarray(xsh0.reshape(cfg.NREP, P))

    wts = np.zeros((P, WCOLS), np.float32)
    for l in range(3):
        wts[:, l * 256:l * 256 + P] = np.asarray(inputs[f"W1_{l}"], np.float32)
        wts[:, l * 256 + P:l * 256 + 2 * P] = np.asarray(inputs[f"W0_{l}"],
                                                         np.float32)
    wts[:, 768:771] = np.asarray(inputs["W1_3"], np.float32)
    wts[:, 771:774] = np.asarray(inputs["W0_3"], np.float32)
    wts = wts.astype(BF16)

    iot = np.tile(np.arange(256, dtype=np.float32)[None, :],
                  (P, 1)).astype(BF16)

    in_maps = []
    for c in range(cfg.NC):
        in_maps.append({
            "xrep0": xrep0,
            "x0sh": np.ascontiguousarray(xsh0[c]),
            "idx16": np.ascontiguousarray(tables["idx16"][c]),
            "rankt": np.ascontiguousarray(tables["rank"][c]).astype(BF16),
            "wts": wts,
            "iot": iot,
        })
    return in_maps


def _np_fallback(inputs):
    x = np.asarray(inputs["features"], np.float32)
    e = np.asarray(inputs["edges"], np.int64)
    src, dst = e[:, 0], e[:, 1]

    def gc(x, i):
        h0 = x @ np.asarray(inputs[f"W0_{i}"], np.float32) + \
            np.asarray(inputs[f"b0_{i}"], np.float32)
        h1 = x @ np.asarray(inputs[f"W1_{i}"], np.float32) + \
            np.asarray(inputs[f"b1_{i}"], np.float32)
        agg = np.zeros_like(h0)
        np.add.at(agg, src, h1[dst])
        np.add.at(agg, dst, h1[src])
        return h0 + agg

    h = np.maximum(gc(x, 0), 0)
    h = np.maximum(gc(h, 1), 0)
    h = np.maximum(gc(h, 2), 0)
    h = h + x
    return gc(h, 3).reshape(2, 4, 40000, 3)


_NC_CACHE = {}
LAST_EXEC_TIME_NS = None


def kernel(**inputs):
    global LAST_EXEC_TIME_NS
    cfg = CFG_FULL
    feats = np.asarray(inputs["features"])
    assert feats.shape == (cfg.N, 128)

    for i in range(4):
        for b in ("b0", "b1"):
            v = inputs.get(f"{b}_{i}")
            if v is not None and np.any(np.asarray(v) != 0):
                return _np_fallback(inputs)

    from concourse.bass_utils import run_bass_kernel_spmd

    edges = np.asarray(inputs["edges"]).astype(np.int64)
    ek = hash(edges.tobytes())
    if _NC_CACHE.get("key") != ek:
        tables = build_tables(edges, cfg)
        _NC_CACHE.update(key=ek, tables=tables,
                         nc=build_nc(cfg, tables))
    nc = _NC_CACHE["nc"]
    tables = _NC_CACHE["tables"]
    in_maps = _pack_inputs(inputs, tables, cfg)

    import os
    trace = bool(int(os.environ.get("GNN_TRACE", "0")))
    res = run_bass_kernel_spmd(nc, in_maps, core_ids=list(range(cfg.NC)),
                               trace=trace)
    LAST_EXEC_TIME_NS = res.exec_time_ns

    vert_at = tables["vert_at"]
    full = np.zeros((cfg.N, 3), np.float32)
    for c in range(cfg.NC):
        o = np.asarray(res.results[c]["out"], np.float32)
        m = vert_at[c] >= 0
        full[vert_at[c][m]] = o[m]
    return full.reshape(2, 4, 40000, 3)


# revision 7
# speedup vs baseline: 1.4358x; 1.0322x over previous
"""MeshRefineNet GNN on 8 TRN2 cores — dma_gather-based aggregation (v2).

v1 gathered neighbor rows with per-tile indirect DMAs (128 descriptors each,
~1us fixed SWDGE cost -> 10ms on GPSIMD).  v2 stages each batch's half-edge
rows with a few large dma_gather calls instead (no per-128-row fixed cost,
~8.5ns/descriptor Q7 generation), which
requires int16 indices: the replica is addressed through 10 windows of 32K
rows, and each batch's staged rows are sorted by (window, group, source row).
Tiles of 128 staged rows span at most 2 destination groups; a bf16 rank table
plus one is_equal builds the 0/1 scatter matrices S (width 256 = 2 group
blocks), and TensorE accumulates X_t^T @ S_t per group in PSUM.  The uniform
(max-over-cores) segment lengths keep the compiled program SPMD-identical;
per-core tables carry the indices/ranks.  Transforms, residual, and the
per-layer AllGather are unchanged from v1.
"""
import sys

if "/opt/trn_rl_repo" not in sys.path:
    sys.path.insert(0, "/opt/trn_rl_repo")

import numpy as np

P = 128
SUBW = 32
GB = 16            # groups per batch
WIN = 32768        # dma_gather int16 window (rows)
MAX_RUN = 8192     # split longer runs (descriptor-ring safety)


class Cfg:
    def __init__(self, N, E, NC=8, GROUPS=313):
        self.N, self.E, self.NC, self.GROUPS = N, E, NC, GROUPS
        self.SLOTS = GROUPS * 128
        self.NREP = NC * self.SLOTS
        self.NWIN = -(-self.NREP // WIN)
        self.NB = -(-GROUPS // GB)
        assert self.NREP >= N


CFG_FULL = Cfg(N=320000, E=960000, NC=8, GROUPS=313)


# ----------------------------------------------------------------- host prep
def build_tables(edges, cfg):
    N, NC, GROUPS = cfg.N, cfg.NC, cfg.GROUPS
    SLOTS, NB, NWIN = cfg.SLOTS, cfg.NB, cfg.NWIN

    src0 = edges[:, 0].astype(np.int64)
    dst0 = edges[:, 1].astype(np.int64)
    tgt = np.concatenate([src0, dst0])
    src = np.concatenate([dst0, src0])
    deg = np.bincount(tgt, minlength=N)

    # balanced vertex->slot assignment (degree snake over 32-row bins)
    NBINS = NC * GROUPS * 4
    n_items = NBINS * SUBW
    deg_ext = np.concatenate([deg, np.zeros(n_items - N, np.int64)])
    order = np.argsort(-deg_ext, kind="stable")
    grid = order.reshape(SUBW, NBINS).copy()
    for r in range(1, SUBW, 2):
        grid[r] = grid[r, ::-1]
    item_ids = grid.ravel()
    rr, bb = np.divmod(np.arange(SUBW * NBINS), NBINS)
    core_of = np.empty(n_items, np.int64)
    slot_of = np.empty(n_items, np.int64)
    core_of[item_ids] = bb // (GROUPS * 4)
    g_item = (bb % (GROUPS * 4)) // 4
    slot_of[item_ids] = g_item * 128 + (bb % 4) * SUBW + rr
    R = core_of * SLOTS + slot_of

    vert_at = np.full((NC, SLOTS), -1, np.int64)
    vert_at[core_of[:N], slot_of[:N]] = np.arange(N)

    c_t = core_of[tgt]
    s_t = slot_of[tgt]
    g_t = s_t // 128
    b_t = g_t // GB
    Rs = R[src]
    w_t = Rs // WIN

    # per (core, batch, window, group) counts -> uniform max lengths
    key = ((c_t * NB + b_t) * NWIN + w_t) * GROUPS + g_t
    cnt = np.bincount(key, minlength=NC * NB * NWIN * GROUPS) \
        .reshape(NC, NB, NWIN, GROUPS)
    Lmax = cnt.max(axis=0)          # [NB, NWIN, GROUPS]

    # ---- uniform walk: tiles / runs / mm schedule (core-independent) ----
    structure = []
    batchoff = []
    total = 0
    segstart = {}
    blockmap = {}                   # (tile_global, g) -> block index
    for b in range(NB):
        glo = b * GB
        ghi = min((b + 1) * GB, GROUPS)
        pos = 0
        runs = []
        tile_groups = []            # batch-local tile -> [g0, g1?]
        mms = []                    # (tile_local, block, g)
        for w in range(NWIN):
            run_start = pos
            for g in range(glo, ghi):
                L = int(Lmax[b, w, g])
                if L == 0:
                    continue
                t = pos // 128
                if pos % 128 != 0:
                    tg = tile_groups[t]
                    if g not in tg and len(tg) == 3:
                        pos = (t + 1) * 128
                segstart[(b, w, g)] = pos
                while L > 0:
                    t = pos // 128
                    while t >= len(tile_groups):
                        tile_groups.append([])
                    tg = tile_groups[t]
                    if g not in tg:
                        assert len(tg) < 3
                        tg.append(g)
                        mms.append((t, tg.index(g), g))
                        blockmap[(total // 128 + t, g)] = tg.index(g)
                    room = (t + 1) * 128 - pos
                    step = min(room, L)
                    pos += step
                    L -= step
            if pos % 128:
                pos = (pos // 128 + 1) * 128
            if pos > run_start:
                runs.append((w, run_start, pos - run_start))
        T_b = pos // 128
        # group-major MM lists (one PSUM bank session per group)
        gmms = {g: [] for g in range(glo, ghi)}
        for (t, blk, g) in mms:
            gmms[g].append((t, blk))
        for g in range(glo, ghi):
            assert gmms[g], f"group {g} empty in batch {b}"
        # compacted S2 slots for tiles holding a 3rd group
        s2loc = {}
        for t, tg in enumerate(tile_groups):
            if len(tg) == 3:
                s2loc[t] = len(s2loc)
        structure.append(dict(T=T_b, runs=runs, gmms=gmms,
                              glo=glo, ng=ghi - glo, s2loc=s2loc,
                              s2off=sum(len(st["s2loc"])
                                        for st in structure)))
        batchoff.append(total)
        total += T_b * 128

    T_total = total // 128
    T2_total = sum(len(st["s2loc"]) for st in structure)

    # ---- per-core tables: staged position of every half-edge ----
    # order within a segment: by (slot, source row)
    seg_of = ((c_t * NB + b_t) * NWIN + w_t) * GROUPS + g_t
    eorder = np.lexsort((Rs, s_t, seg_of))
    so = seg_of[eorder]
    first_in_seg = np.searchsorted(so, np.arange(NC * NB * NWIN * GROUPS))
    cum = np.arange(so.size) - first_in_seg[so]

    segstart_arr = np.zeros((NB, NWIN, GROUPS), np.int64)
    for (b, w, g), v in segstart.items():
        segstart_arr[b, w, g] = v
    boff_arr = np.asarray(batchoff, np.int64)

    et = eorder
    pos_local = segstart_arr[b_t[et], w_t[et], g_t[et]] + cum
    pos_global = boff_arr[b_t[et]] + pos_local
    tile_g = pos_global // 128
    part_g = pos_global % 128

    blk_of = np.zeros(et.size, np.int64)
    bm_keys = np.array([k[0] * GROUPS + k[1] for k in blockmap], np.int64)
    bm_vals = np.array(list(blockmap.values()), np.int64)
    bm = np.zeros(T_total * GROUPS, np.int64)
    bm[bm_keys] = bm_vals
    blk_of = bm[tile_g * GROUPS + g_t[et]]

    idx_lin = np.zeros((NC, total), np.int16)
    rank_lin = np.full((NC, total), 512.0, np.float32)
    rank2_lin = np.full((NC, total), 512.0, np.float32)
    cc = c_t[et]
    idx_lin[cc, pos_global] = (Rs[et] % WIN).astype(np.int16)
    r1v = np.where(blk_of < 2, s_t[et] % 128 + 128 * blk_of, 512)
    r2v = np.where(blk_of == 2, s_t[et] % 128, 512)
    rank_lin[cc, pos_global] = r1v.astype(np.float32)
    rank2_lin[cc, pos_global] = r2v.astype(np.float32)

    # wrapped idx layout per run: local i -> (partition i%16 (+16k), col i//16)
    idx16 = np.zeros((NC, 128, total // 16), np.int16)
    for b in range(NB):
        for (w, start, L) in structure[b]["runs"]:
            s0 = batchoff[b] + start
            blkv = idx_lin[:, s0:s0 + L].reshape(NC, L // 16, 16) \
                .transpose(0, 2, 1)                       # [NC, 16, L/16]
            idx16[:, :, s0 // 16:(s0 + L) // 16] = np.tile(blkv, (1, 8, 1))

    rank_tbl = rank_lin.reshape(NC, T_total, 128).transpose(0, 2, 1).copy()
    rank2_full = rank2_lin.reshape(NC, T_total, 128).transpose(0, 2, 1)
    rank2_tbl = np.full((NC, 128, max(T2_total, 1)), 512.0, np.float32)
    for b, st in enumerate(structure):
        bt0 = batchoff[b] // 128
        for t, p2 in st["s2loc"].items():
            rank2_tbl[:, :, st["s2off"] + p2] = rank2_full[:, :, bt0 + t]

    return dict(idx16=idx16, rank=rank_tbl, rank2=rank2_tbl,
                vert_at=vert_at,
                structure=structure, batchoff=batchoff, T_total=T_total,
                T2_total=T2_total,
                T_max=max(s["T"] for s in structure),
                T2_max=max(len(s["s2loc"]) for s in structure))


def permute_rows(x, vert_at, cfg):
    out = np.zeros((cfg.NC, cfg.SLOTS, x.shape[1]), x.dtype)
    m = vert_at >= 0
    out[m] = x[vert_at[m]]
    return out


# ------------------------------------------------------------- device build
def build_nc(cfg, tables):
    import concourse.bacc as bacc
    import concourse.tile as tile
    import concourse.mybir as mybir
    from concourse import library_config

    BF = mybir.dt.bfloat16
    F32 = mybir.dt.float32
    I16 = mybir.dt.int16
    RELU = mybir.ActivationFunctionType.Relu
    WCOLS = 6 * P + 6

    NC_, GROUPS, SLOTS, NREP = cfg.NC, cfg.GROUPS, cfg.SLOTS, cfg.NREP
    structure = tables["structure"]
    batchoff = tables["batchoff"]
    T_total = tables["T_total"]
    T_max = tables["T_max"]
    HALF = -(-T_max // 2)

    nc = bacc.Bacc(None, target_bir_lowering=False, debug=False)
    xrep0 = nc.declare_dram_parameter("xrep0", [NREP, P], BF, isOutput=False)
    x0sh = nc.declare_dram_parameter("x0sh", [SLOTS, P], BF, isOutput=False)
    idx16 = nc.declare_dram_parameter("idx16", [P, T_total * 8], I16,
                                      isOutput=False)
    rankt = nc.declare_dram_parameter("rankt", [P, T_total], BF,
                                      isOutput=False)
    T2_total = max(tables["T2_total"], 1)
    T2_max = max(tables["T2_max"], 1)
    rankt2 = nc.declare_dram_parameter("rankt2", [P, T2_total], BF,
                                       isOutput=False)
    wts = nc.declare_dram_parameter("wts", [P, WCOLS], BF, isOutput=False)
    iot = nc.declare_dram_parameter("iot", [P, 256], BF, isOutput=False)
    out = nc.declare_dram_parameter("out", [SLOTS, 3], F32, isOutput=True)
    import os as _os
    DBG = bool(int(_os.environ.get("GNN_DBG", "0")))
    if DBG:
        T0 = tables["structure"][0]["T"]
        dbg_stg = nc.declare_dram_parameter("dbg_stg", [P, T0 * P], BF,
                                            isOutput=True)
        dbg_s = nc.declare_dram_parameter("dbg_s", [P, T0 * 256], BF,
                                          isOutput=True)


    xsh = {l: nc.dram_tensor(f"xsh{l}", [SLOTS, P], BF) for l in (1, 2, 3)}
    xrep = {l: nc.dram_tensor(f"xrep{l}", [NREP, P], BF, addr_space="Shared")
            for l in (1, 2, 3)}

    with tile.TileContext(nc) as tc:
        with (
            tc.tile_pool(name="res", bufs=1) as res,
            tc.tile_pool(name="idx", bufs=2) as idx_p,
            tc.tile_pool(name="stg", bufs=2) as stg_p,
            tc.tile_pool(name="s", bufs=2) as s_p,
            tc.tile_pool(name="xt", bufs=2) as xt_p,
            tc.tile_pool(name="x0t", bufs=2) as x0t_p,
            tc.tile_pool(name="xat", bufs=4) as xat_p,
            tc.tile_pool(name="ob", bufs=2) as ob_p,
            tc.tile_pool(name="psA", bufs=6, space="PSUM") as psA,
            tc.tile_pool(name="psB", bufs=2, space="PSUM") as psB,
        ):
            nc.gpsimd.load_library(library_config.attnmlp)
            rank_sb = res.tile([P, T_total], BF)
            nc.sync.dma_start(out=rank_sb[:], in_=rankt.ap())
            rank2_sb = res.tile([P, T2_total], BF)
            nc.sync.dma_start(out=rank2_sb[:], in_=rankt2.ap())
            iota_sb = res.tile([P, HALF * 256], BF)
            nc.sync.dma_start(
                out=iota_sb[:].rearrange("p (t c) -> p t c", c=256),
                in_=iot.ap().rearrange("p (o c) -> p o c", o=1)
                    .to_broadcast([P, HALF, 256]))
            wts_sb = res.tile([P, WCOLS], BF)
            nc.sync.dma_start(out=wts_sb[:], in_=wts.ap())

            for layer in range(4):
                rep_ap = xrep0.ap() if layer == 0 else xrep[layer].ap()
                sh_ap = x0sh.ap() if layer == 0 else xsh[layer].ap()
                if layer < 3:
                    w1 = wts_sb[:, layer * 256: layer * 256 + P]
                    w0 = wts_sb[:, layer * 256 + P: layer * 256 + 2 * P]
                else:
                    w1 = wts_sb[:, 768:771]
                    w0 = wts_sb[:, 771:774]

                for b, st in enumerate(structure):
                    T_b, ng, glo = st["T"], st["ng"], st["glo"]
                    boff = batchoff[b]
                    bt0 = boff // 128

                    idxb = idx_p.tile([P, T_max * 8], I16, tag="idx")
                    nc.sync.dma_start(
                        out=idxb[:, 0:T_b * 8],
                        in_=idx16.ap()[:, boff // 16:(boff + T_b * 128) // 16])
                    stg = stg_p.tile([P, T_max, P], BF, tag="stg")
                    for (w, start, L) in st["runs"]:
                        w_hi = min((w + 1) * WIN, NREP)
                        o = 0
                        while o < L:
                            Lc = min(MAX_RUN, L - o)
                            nc.gpsimd.dma_gather(
                                out_ap=stg[:, (start + o) // 128:
                                           (start + o + Lc) // 128, :],
                                in_ap=rep_ap[w * WIN:w_hi, :],
                                idxs_ap=idxb[:, (start + o) // 16:
                                             (start + o + Lc) // 16],
                                num_idxs=Lc,
                                num_idxs_reg=Lc,
                                elem_size=P,
                                single_packet=False,
                            )
                            o += Lc

                    s_tiles = []
                    for hb in range(2):
                        t0 = hb * HALF
                        t1 = min(T_b, (hb + 1) * HALF)
                        if t0 >= t1:
                            s_tiles.append(None)
                            continue
                        S = s_p.tile([P, HALF * 256], BF, tag="s")
                        nc.vector.tensor_tensor(
                            out=S[:, 0:(t1 - t0) * 256]
                                .rearrange("p (t c) -> p t c", c=256),
                            in0=rank_sb[:, bt0 + t0:bt0 + t1]
                                .rearrange("p (t o) -> p t o", o=1)
                                .to_broadcast([P, t1 - t0, 256]),
                            in1=iota_sb[:, 0:(t1 - t0) * 256]
                                .rearrange("p (t c) -> p t c", c=256),
                            op=mybir.AluOpType.is_equal,
                        )
                        s_tiles.append((S, t0))

                    T2_b = len(st["s2loc"])
                    s2l = st["s2loc"]
                    if T2_b:
                        S2b = s_p.tile([P, T2_max * P], BF, tag="s2")
                        nc.vector.tensor_tensor(
                            out=S2b[:, 0:T2_b * P]
                                .rearrange("p (t c) -> p t c", c=P),
                            in0=rank2_sb[:, st["s2off"]:st["s2off"] + T2_b]
                                .rearrange("p (t o) -> p t o", o=1)
                                .to_broadcast([P, T2_b, P]),
                            in1=iota_sb[:, 0:T2_b * 256]
                                .rearrange("p (t c) -> p t c", c=256)
                                [:, :, 0:P],
                            op=mybir.AluOpType.is_equal,
                        )

                    if DBG and layer == 0 and b == 0:
                        nc.sync.dma_start(
                            out=dbg_stg.ap(),
                            in_=stg[:, 0:T_b, :].rearrange("p a b -> p (a b)"))
                        for hb in range(2):
                            if s_tiles[hb] is None:
                                continue
                            S, t0 = s_tiles[hb]
                            t1 = min(T_b, t0 + HALF)
                            nc.sync.dma_start(
                                out=dbg_s.ap()[:, t0 * 256:t1 * 256],
                                in_=S[:, 0:(t1 - t0) * 256])

                    # ---- per-group aggregation + transforms ----
                    g0 = glo
                    xt = xt_p.tile([P, GB * P], BF, tag="xt")
                    nc.sync.dma_start(out=xt[:, 0:ng * P],
                                      in_=sh_ap[g0 * P:(g0 + ng) * P, :],
                                      transpose=True)
                    if layer == 2:
                        x0v = x0t_p.tile([P, GB, P], BF, tag="x0v")
                        nc.sync.dma_start(
                            out=x0v[:, 0:ng, :],
                            in_=x0sh.ap()[g0 * P:(g0 + ng) * P, :]
                                .rearrange("(g p) c -> p g c", p=P))
                    if layer < 3:
                        obuf = ob_p.tile([P, GB, P], BF, tag="ob")
                    else:
                        obuf = ob_p.tile([P, GB, 3], F32, tag="ob3")

                    for gl in range(ng):
                        glist = st["gmms"][glo + gl]
                        pg = psA.tile([P, P], F32, tag="pg")
                        for i, (t, blk) in enumerate(glist):
                            if blk < 2:
                                S, t0 = s_tiles[0] if t < HALF else s_tiles[1]
                                rhs = S[:, (t - t0) * 256 + blk * 128:
                                        (t - t0) * 256 + blk * 128 + 128]
                            else:
                                rhs = S2b[:, s2l[t] * P:(s2l[t] + 1) * P]
                            nc.tensor.matmul(
                                pg[:],
                                lhsT=stg[:, t, :],
                                rhs=rhs,
                                start=(i == 0), stop=(i == len(glist) - 1),
                            )
                        xat = xat_p.tile([P, P], BF)
                        nc.vector.tensor_copy(out=xat[:], in_=pg[:])
                        if layer < 3:
                            ps2 = psB.tile([P, P], F32, tag="ps2")
                            nc.tensor.matmul(ps2[:], lhsT=xat[:], rhs=w1,
                                             start=True, stop=False)
                            nc.tensor.matmul(ps2[:],
                                             lhsT=xt[:, gl * P:(gl + 1) * P],
                                             rhs=w0, start=False, stop=True)
                            if layer == 2:
                                rl = xat_p.tile([P, P], BF, tag="rl")
                                nc.scalar.activation(out=rl[:], in_=ps2[:],
                                                     func=RELU)
                                nc.vector.tensor_add(out=obuf[:, gl, :],
                                                     in0=rl[:],
                                                     in1=x0v[:, gl, :])
                            else:
                                nc.scalar.activation(out=obuf[:, gl, :],
                                                     in_=ps2[:], func=RELU)
                        else:
                            ps2 = psB.tile([P, P], F32, tag="ps2")
                            nc.tensor.matmul(ps2[:, 0:3], lhsT=xat[:], rhs=w1,
                                             start=True, stop=False)
                            nc.tensor.matmul(ps2[:, 0:3],
                                             lhsT=xt[:, gl * P:(gl + 1) * P],
                                             rhs=w0, start=False, stop=True)
                            nc.vector.tensor_copy(out=obuf[:, gl, :],
                                                  in_=ps2[:, 0:3])

                    if layer < 3:
                        dst = xsh[layer + 1].ap()[g0 * P:(g0 + ng) * P, :] \
                            .rearrange("(g p) c -> p g c", p=P)
                    else:
                        dst = out.ap()[g0 * P:(g0 + ng) * P, :] \
                            .rearrange("(g p) c -> p g c", p=P)
                    nc.sync.dma_start(out=dst, in_=obuf[:, 0:ng, :])

                if layer < 3:
                    nc.gpsimd.collective_compute(
                        "AllGather", mybir.AluOpType.bypass,
                        replica_groups=[list(range(NC_))],
                        ins=[xsh[layer + 1].ap().opt()],
                        outs=[xrep[layer + 1].ap().opt()],
                    )
    nc.compile()
    return nc


# --------------------------------------------------------------- host driver
def _pack_inputs(inputs, tables, cfg):
    import ml_dtypes
    BF16 = ml_dtypes.bfloat16
    WCOLS = 6 * P + 6

    feats = np.asarray(inputs["features"], np.float32)
    xsh0 = permute_rows(feats, tables["vert_at"], cfg).astype(BF16)
    xrep0 = np.ascontiguousarray(xsh0.reshape(cfg.NREP, P))

    wts = np.zeros((P, WCOLS), np.float32)
    for l in range(3):
        wts[:, l * 256:l * 256 + P] = np.asarray(inputs[f"W1_{l}"], np.float32)
        wts[:, l * 256 + P:l * 256 + 2 * P] = np.asarray(inputs[f"W0_{l}"],
                                                         np.float32)
    wts[:, 768:771] = np.asarray(inputs["W1_3"], np.float32)
    wts[:, 771:774] = np.asarray(inputs["W0_3"], np.float32)
    wts = wts.astype(BF16)

    iot = np.tile(np.arange(256, dtype=np.float32)[None, :],
                  (P, 1)).astype(BF16)

    in_maps = []
    for c in range(cfg.NC):
        in_maps.append({
            "xrep0": xrep0,
            "x0sh": np.ascontiguousarray(xsh0[c]),
            "idx16": np.ascontiguousarray(tables["idx16"][c]),
            "rankt": np.ascontiguousarray(tables["rank"][c]).astype(BF16),
            "rankt2": np.ascontiguousarray(tables["rank2"][c]).astype(BF16),
            "wts": wts,
            "iot": iot,
        })
    return in_maps


def _np_fallback(inputs):
    x = np.asarray(inputs["features"], np.float32)
    e = np.asarray(inputs["edges"], np.int64)
    src, dst = e[:, 0], e[:, 1]

    def gc(x, i):
        h0 = x @ np.asarray(inputs[f"W0_{i}"], np.float32) + \
            np.asarray(inputs[f"b0_{i}"], np.float32)
        h1 = x @ np.asarray(inputs[f"W1_{i}"], np.float32) + \
            np.asarray(inputs[f"b1_{i}"], np.float32)
        agg = np.zeros_like(h0)
        np.add.at(agg, src, h1[dst])
        np.add.at(agg, dst, h1[src])
        return h0 + agg

    h = np.maximum(gc(x, 0), 0)
    h = np.maximum(gc(h, 1), 0)
    h = np.maximum(gc(h, 2), 0)
    h = h + x
    return gc(h, 3).reshape(2, 4, 40000, 3)


_NC_CACHE = {}
LAST_EXEC_TIME_NS = None


def kernel(**inputs):
    global LAST_EXEC_TIME_NS
    cfg = CFG_FULL
    feats = np.asarray(inputs["features"])
    assert feats.shape == (cfg.N, 128)

    for i in range(4):
        for b in ("b0", "b1"):
            v = inputs.get(f"{b}_{i}")
            if v is not None and np.any(np.asarray(v) != 0):
                return _np_fallback(inputs)

    from concourse.bass_utils import run_bass_kernel_spmd

    edges = np.asarray(inputs["edges"]).astype(np.int64)
    ek = hash(edges.tobytes())
    if _NC_CACHE.get("key") != ek:
        tables = build_tables(edges, cfg)
        _NC_CACHE.update(key=ek, tables=tables,
                         nc=build_nc(cfg, tables))
    nc = _NC_CACHE["nc"]
    tables = _NC_CACHE["tables"]
    in_maps = _pack_inputs(inputs, tables, cfg)

    import os
    trace = bool(int(os.environ.get("GNN_TRACE", "0")))
    res = run_bass_kernel_spmd(nc, in_maps, core_ids=list(range(cfg.NC)),
                               trace=trace)
    LAST_EXEC_TIME_NS = res.exec_time_ns

    vert_at = tables["vert_at"]
    full = np.zeros((cfg.N, 3), np.float32)
    for c in range(cfg.NC):
        o = np.asarray(res.results[c]["out"], np.float32)
        m = vert_at[c] >= 0
        full[vert_at[c][m]] = o[m]
    return full.reshape(2, 4, 40000, 3)
